# revision 12
# baseline (speedup 1.0000x reference)
"""Trainium2 Bass kernel for nn_AssociationCortex (MoE routing block).

Math (per reference):
  x = [dorsal | ventral]                    (B, C)   B=8192, C=2048
  gate_logits = x @ gate_W                  (B, E)   E=8
  top-2 softmax -> gate_weights             (B, E)  (2 nonzeros/row)
  h_e = gelu(x @ W1_e + b1_e)               (B, ED)  ED=1024, all 8 experts
  out_e = h_e @ W2_e + b2_e                 (B, ED)
  bound = sum_e g_e * out_e                 (B, ED)
  assoc = bound @ Wo + bo                   (B, OD)
  fb_d = 0.5 * assoc @ Wfd ; fb_v = 0.5 * assoc @ Wfv
  returns (assoc, fb_d, fb_v, gate_weights)

Strategy: data-parallel over batch across 8 NeuronCores (1024 rows/core,
all params replicated; no collectives). On-device layout is transposed
(features on partitions, batch on the free axis) so every matmul uses the
natural weight block as the stationary operand and the activation as the
moving operand. Compute in bf16 (fp32 PSUM accumulation); gate logits use
a bf16 hi+lo decomposition (x_hi@g_hi + x_hi@g_lo + x_lo@g_hi) for ~fp32
accuracy, since top-k selection is discretely sensitive to logit error.
Gate weighting of h is done with a PE row-broadcast of the dense gate row
followed by a VectorE multiply. All host-side work is layout prep only
(transpose/cast/shard); every FLOP of the model runs on device.
"""

import sys

if "/opt/trn_rl_repo" not in sys.path:
    sys.path.insert(0, "/opt/trn_rl_repo")

import numpy as np
import ml_dtypes

import concourse.bass as bass
import concourse.mybir as mybir
import concourse.tile as tile
from concourse import bacc
from concourse.masks import make_identity

BF16 = mybir.dt.bfloat16
F32 = mybir.dt.float32
AF = mybir.ActivationFunctionType
ALU = mybir.AluOpType
bf16 = ml_dtypes.bfloat16

# Problem dims (hardcoded per spec)
B, D, V, E, ED, OD = 8192, 1024, 1024, 8, 1024, 1024
C = D + V
NCORES = 8
N = B // NCORES          # batch rows per core (1024)
BC = 512                 # batch chunk (matmul moving free dim)
NJ = N // BC             # chunks per core (2)
CT = C // 128            # 16 c tiles
DT8 = ED // 128          # 8 feature tiles
FB_STRENGTH = 0.5


def build_nc():
    nc = bacc.Bacc("TRN2", target_bir_lowering=False, debug=False, num_devices=NCORES)

    # ---- DRAM parameters (per-core shards / replicated) ----
    xt_hi = nc.declare_dram_parameter("xt_hi", [C, N], BF16, isOutput=False)
    xt_lo = nc.declare_dram_parameter("xt_lo", [C, N], BF16, isOutput=False)
    gw_hi = nc.declare_dram_parameter("gw_hi", [C, E], BF16, isOutput=False)
    gw_lo = nc.declare_dram_parameter("gw_lo", [C, E], BF16, isOutput=False)
    # w1[e, dblk, p, co, f] = W1[e, co*128+p, dblk*128+f]
    w1 = nc.declare_dram_parameter("w1", [E, DT8, 128, CT, 128], BF16, isOutput=False)
    # w2[e, fblk, p, do, f] = W2[e, do*128+p, fblk*128+f]
    w2 = nc.declare_dram_parameter("w2", [E, DT8, 128, DT8, 128], BF16, isOutput=False)
    # b1r[e, p, t] = b1[e, t*128+p]
    b1r = nc.declare_dram_parameter("b1r", [E, 128, DT8], F32, isOutput=False)
    b2n = nc.declare_dram_parameter("b2n", [E, ED], BF16, isOutput=False)
    # wo3[oblk, p, fo, o2] = Wo[fo*128+p, oblk*128+o2]; same scheme for wfd/wfv
    wo3 = nc.declare_dram_parameter("wo3", [DT8, 128, DT8, 128], BF16, isOutput=False)
    wfd3 = nc.declare_dram_parameter("wfd3", [DT8, 128, DT8, 128], BF16, isOutput=False)
    wfv3 = nc.declare_dram_parameter("wfv3", [DT8, 128, DT8, 128], BF16, isOutput=False)
    # bor[p, t] = bo[t*128+p]
    bor = nc.declare_dram_parameter("bor", [128, DT8], F32, isOutput=False)

    assocT = nc.declare_dram_parameter("assocT", [OD, N], F32, isOutput=True)
    fbdT = nc.declare_dram_parameter("fbdT", [D, N], F32, isOutput=True)
    fbvT = nc.declare_dram_parameter("fbvT", [V, N], F32, isOutput=True)
    gatew = nc.declare_dram_parameter("gatew", [N, E], F32, isOutput=True)

    from contextlib import ExitStack

    with ExitStack() as ctx:
        tc = ctx.enter_context(tile.TileContext(nc))
        pool = lambda name, bufs, **kw: ctx.enter_context(  # noqa: E731
            tc.tile_pool(name=name, bufs=bufs, **kw))
        constp = pool("const", 1)
        xhp = pool("xh", CT)
        xlp = pool("xl", 6)
        gwp = pool("gw", 2 * CT)
        gsp = pool("gsmall", 1)
        gnp = pool("gnat", 24)
        w1p = pool("w1d", 5)
        w2p = pool("w2f", 5)
        wfp = pool("wfin", 4)
        biasp = pool("bias", 4)
        gbp = pool("gb", 3)
        hbp = pool("hbf", 4)
        hgp = pool("hg", 34)
        boundp = pool("bound", 2 * DT8)
        evp = pool("evf32", 4)
        php = pool("ps_h", 3, space="PSUM")
        pzp = pool("ps_z", 3, space="PSUM")
        pmp = pool("ps_misc", 2, space="PSUM")
        if True:
            # ---- constants ----
            ident = constp.tile([128, 128], F32)
            make_identity(nc, ident[:])
            ones_bf = constp.tile([1, 128], BF16)
            nc.vector.memset(ones_bf[:], 1.0)

            # ---- load x^T (hi) tiles: persistent through the expert loop ----
            xh = []
            for c in range(CT):
                t = xhp.tile([128, N], BF16, tag="xh")
                nc.sync.dma_start(t[:], xt_hi[c * 128 : (c + 1) * 128, :])
                xh.append(t)

            # ---- gate logits, hi/lo decomposed, accumulated in PSUM ----
            # glog_ps[j] (8, BC) += gwh[c].T@xh + gwl[c].T@xh + gwh[c].T@xl
            glog_ps = [pmp.tile([E, BC], F32, tag="pm", name=f"glog{_}") for _ in range(NJ)]
            gwh, gwl = [], []
            for c in range(CT):
                th = gwp.tile([128, E], BF16, tag="gwh")
                nc.sync.dma_start(th[:], gw_hi[c * 128 : (c + 1) * 128, :])
                gwh.append(th)
                tl = gwp.tile([128, E], BF16, tag="gwl")
                nc.sync.dma_start(tl[:], gw_lo[c * 128 : (c + 1) * 128, :])
                gwl.append(tl)
            for c in range(CT):
                xlt = xlp.tile([128, N], BF16, tag="xl")
                nc.sync.dma_start(xlt[:], xt_lo[c * 128 : (c + 1) * 128, :])
                first, last = c == 0, c == CT - 1
                # same stationary serves adjacent matmuls (LDW reuse)
                for j in range(NJ):
                    bs = slice(j * BC, (j + 1) * BC)
                    nc.tensor.matmul(glog_ps[j][:], gwh[c][:], xh[c][:, bs],
                                     start=first, stop=False)
                for j in range(NJ):
                    bs = slice(j * BC, (j + 1) * BC)
                    nc.tensor.matmul(glog_ps[j][:], gwh[c][:], xlt[:, bs],
                                     start=False, stop=False)
                for j in range(NJ):
                    bs = slice(j * BC, (j + 1) * BC)
                    nc.tensor.matmul(glog_ps[j][:], gwl[c][:], xh[c][:, bs],
                                     start=False, stop=last)

            g_sb = gsp.tile([E, N], F32)
            for j in range(NJ):
                nc.vector.tensor_copy(g_sb[:, j * BC : (j + 1) * BC], glog_ps[j][:])

            # ---- top-2 + softmax, in natural layout (batch on partitions) ----
            gw_nat = []
            for t in range(N // 128):
                tr_ps = pmp.tile([128, E], F32, tag="pm")
                nc.tensor.transpose(tr_ps[:], g_sb[:, t * 128 : (t + 1) * 128],
                                    ident[:E, :E])
                gn = gnp.tile([128, E], F32, tag="gn")
                nc.vector.tensor_copy(gn[:], tr_ps[:])

                m1 = gnp.tile([128, 1], F32, tag="m")
                nc.vector.tensor_reduce(m1[:], gn[:], mybir.AxisListType.X, ALU.max)
                mask1 = gnp.tile([128, E], F32, tag="gn")
                nc.vector.tensor_scalar(mask1[:], gn[:], m1[:], None, ALU.is_equal)
                masked = gnp.tile([128, E], F32, tag="gn")
                nc.vector.scalar_tensor_tensor(masked[:], mask1[:], -1e30, gn[:],
                                               ALU.mult, ALU.add)
                m2 = gnp.tile([128, 1], F32, tag="m")
                nc.vector.tensor_reduce(m2[:], masked[:], mybir.AxisListType.X, ALU.max)
                mask2 = gnp.tile([128, E], F32, tag="gn")
                nc.vector.tensor_scalar(mask2[:], masked[:], m2[:], None, ALU.is_equal)
                negm2 = gnp.tile([128, 1], F32, tag="m")
                nc.vector.tensor_scalar_mul(negm2[:], m2[:], -1.0)
                w1v = gnp.tile([128, 1], F32, tag="m")
                nc.scalar.activation(w1v[:], m1[:], AF.Sigmoid, bias=negm2[:])
                w2v = gnp.tile([128, 1], F32, tag="m")
                nc.vector.tensor_scalar(w2v[:], w1v[:], -1.0, 1.0, ALU.mult, ALU.add)

                gwn = gnp.tile([128, E], F32, tag="gn")
                nc.vector.tensor_scalar(gwn[:], mask2[:], w2v[:], None, ALU.mult)
                nc.vector.scalar_tensor_tensor(gwn[:], mask1[:], w1v[:], gwn[:],
                                               ALU.mult, ALU.add)
                nc.sync.dma_start(gatew[t * 128 : (t + 1) * 128, :], gwn[:])
                gw_nat.append(gwn)

            # ---- dense gate rows g^T (E, N) in bf16 ----
            gT_bf = gsp.tile([E, N], BF16)
            for j in range(NJ):
                gT_ps = pmp.tile([E, BC], F32, tag="pm")
                for k in range(BC // 128):
                    t = j * (BC // 128) + k
                    nc.tensor.transpose(gT_ps[:, k * 128 : (k + 1) * 128],
                                        gw_nat[t][:], ident[:])
                nc.vector.tensor_copy(gT_bf[:, j * BC : (j + 1) * BC], gT_ps[:])

            # per-expert gate rows at partition 0 (matmul rhs needs base 0)
            gTrow = []
            for e in range(E):
                gr = gsp.tile([1, N], BF16, tag=f"gtr{e}", name=f"gtr{e}")
                nc.sync.dma_start(gr[:], gT_bf[e : e + 1, :])
                gTrow.append(gr)

            # ---- seed bound with sum_e g_e * b2_e  (PE over e-dim) ----
            b2sb = gsp.tile([E, ED], BF16)
            nc.sync.dma_start(b2sb[:], b2n[:, :])
            bound = [[None] * DT8 for _ in range(NJ)]
            for f in range(DT8):
                sps = [pzp.tile([128, BC], F32, tag="pz", name=f"sps{f}_{j}")
                       for j in range(NJ)]
                for j in range(NJ):
                    bs = slice(j * BC, (j + 1) * BC)
                    nc.tensor.matmul(sps[j][:], b2sb[:, f * 128 : (f + 1) * 128],
                                     gT_bf[:, bs], start=True, stop=True)
                for j in range(NJ):
                    bt = boundp.tile([128, BC], F32, tag="bound",
                                     name=f"bound{f}_{j}")
                    nc.vector.tensor_copy(bt[:], sps[j][:])
                    bound[j][f] = bt

            # ---- expert loop ----
            for e in range(E):
                # broadcast gate row e -> (128, N) bf16
                gb = gbp.tile([128, N], BF16, tag="gb")
                for j in range(NJ):
                    gb_ps = pmp.tile([128, BC], F32, tag="pm")
                    nc.tensor.matmul(gb_ps[:], ones_bf[:],
                                     gTrow[e][:, j * BC : (j + 1) * BC],
                                     start=True, stop=True)
                    nc.vector.tensor_copy(gb[:, j * BC : (j + 1) * BC], gb_ps[:])

                b1sb = biasp.tile([128, DT8], F32, tag="b1")
                nc.sync.dma_start(b1sb[:], b1r[e, :, :])

                hg = [[None] * NJ for _ in range(DT8)]
                for d in range(DT8):
                    w1t = w1p.tile([128, CT, 128], BF16, tag="w1")
                    nc.sync.dma_start(w1t[:], w1[e, d, :, :, :])
                    ph = [php.tile([128, BC], F32, tag="ph", name=f"ph{e}_{d}_{j}")
                          for j in range(NJ)]
                    for c in range(CT):
                        for j in range(NJ):
                            bs = slice(j * BC, (j + 1) * BC)
                            nc.tensor.matmul(ph[j][:], w1t[:, c, :], xh[c][:, bs],
                                             start=(c == 0), stop=(c == CT - 1))
                    for j in range(NJ):
                        bs = slice(j * BC, (j + 1) * BC)
                        hb = hbp.tile([128, BC], BF16, tag="hb")
                        nc.scalar.activation(hb[:], ph[j][:], AF.Gelu,
                                             bias=b1sb[:, d : d + 1])
                        ht = hgp.tile([128, BC], BF16, tag="hg")
                        nc.vector.tensor_mul(ht[:], hb[:], gb[:, bs])
                        hg[d][j] = ht

                for f in range(DT8):
                    w2t = w2p.tile([128, DT8, 128], BF16, tag="w2")
                    nc.sync.dma_start(w2t[:], w2[e, f, :, :, :])
                    pz = [pzp.tile([128, BC], F32, tag="pz", name=f"pz{e}_{f}_{j}")
                          for j in range(NJ)]
                    for d in range(DT8):
                        for j in range(NJ):
                            nc.tensor.matmul(pz[j][:], w2t[:, d, :], hg[d][j][:],
                                             start=(d == 0), stop=(d == DT8 - 1))
                    for j in range(NJ):
                        nc.vector.tensor_add(bound[j][f][:], pz[j][:], bound[j][f][:])

            # ---- final projections ----
            borsb = biasp.tile([128, DT8], F32, tag="bo")
            nc.sync.dma_start(borsb[:], bor[:, :])

            bound_bf = [[None] * DT8 for _ in range(NJ)]
            for j in range(NJ):
                for f in range(DT8):
                    bb = hgp.tile([128, BC], BF16, tag="hg")
                    nc.vector.tensor_copy(bb[:], bound[j][f][:])
                    bound_bf[j][f] = bb

            assoc_bf = [[None] * NJ for _ in range(DT8)]
            for o in range(DT8):
                wot = wfp.tile([128, DT8, 128], BF16, tag="wf")
                nc.sync.dma_start(wot[:], wo3[o, :, :, :])
                pa = [pzp.tile([128, BC], F32, tag="pz", name=f"pa{o}_{j}")
                      for j in range(NJ)]
                for f in range(DT8):
                    for j in range(NJ):
                        nc.tensor.matmul(pa[j][:], wot[:, f, :], bound_bf[j][f][:],
                                         start=(f == 0), stop=(f == DT8 - 1))
                for j in range(NJ):
                    af = evp.tile([128, BC], F32, tag="ev")
                    nc.vector.tensor_scalar(af[:], pa[j][:], borsb[:, o : o + 1],
                                            None, ALU.add)
                    ab = hgp.tile([128, BC], BF16, tag="hg")
                    nc.vector.tensor_copy(ab[:], af[:])
                    assoc_bf[o][j] = ab
                    nc.sync.dma_start(
                        assocT[o * 128 : (o + 1) * 128, j * BC : (j + 1) * BC], af[:])

            for wi, (wext, out_ext) in enumerate(((wfd3, fbdT), (wfv3, fbvT))):
                for dd in range(DT8):
                    wt = wfp.tile([128, DT8, 128], BF16, tag="wf")
                    nc.sync.dma_start(wt[:], wext[dd, :, :, :])
                    pf = [pzp.tile([128, BC], F32, tag="pz",
                                   name=f"pf{wi}_{dd}_{j}")
                          for j in range(NJ)]
                    for o in range(DT8):
                        for j in range(NJ):
                            nc.tensor.matmul(pf[j][:], wt[:, o, :],
                                             assoc_bf[o][j][:],
                                             start=(o == 0), stop=(o == DT8 - 1))
                    for j in range(NJ):
                        ff = evp.tile([128, BC], F32, tag="ev")
                        nc.vector.tensor_scalar_mul(ff[:], pf[j][:], FB_STRENGTH)
                        nc.sync.dma_start(
                            out_ext[dd * 128 : (dd + 1) * 128,
                                    j * BC : (j + 1) * BC], ff[:])

    nc.compile()
    return nc


_NC_CACHE = []


def _get_nc():
    if not _NC_CACHE:
        _NC_CACHE.append(build_nc())
    return _NC_CACHE[0]


def _split_hi_lo(a32):
    hi = a32.astype(bf16)
    lo = (a32 - hi.astype(np.float32)).astype(bf16)
    return hi, lo


def prepare_in_maps(dorsal, ventral, gate_W, W1, b1, W2, b2, Wo, bo, Wfd, Wfv):
    f32 = np.float32
    x = np.concatenate([np.asarray(dorsal, f32), np.asarray(ventral, f32)], axis=1)
    xT = np.ascontiguousarray(x.T)  # (C, B)
    xt_hi, xt_lo = _split_hi_lo(xT)
    gw_hi, gw_lo = _split_hi_lo(np.asarray(gate_W, f32))

    W1 = np.asarray(W1, f32)
    W2 = np.asarray(W2, f32)
    w1_dev = np.ascontiguousarray(
        W1.reshape(E, CT, 128, DT8, 128).transpose(0, 3, 2, 1, 4)).astype(bf16)
    w2_dev = np.ascontiguousarray(
        W2.reshape(E, DT8, 128, DT8, 128).transpose(0, 3, 2, 1, 4)).astype(bf16)
    b1r = np.ascontiguousarray(
        np.asarray(b1, f32).reshape(E, DT8, 128).transpose(0, 2, 1))
    b2n = np.asarray(b2, f32).astype(bf16)

    def fin(w):
        return np.ascontiguousarray(
            np.asarray(w, f32).reshape(DT8, 128, DT8, 128).transpose(2, 1, 0, 3)
        ).astype(bf16)

    wo3, wfd3, wfv3 = fin(Wo), fin(Wfd), fin(Wfv)
    bor = np.ascontiguousarray(np.asarray(bo, f32).reshape(DT8, 128).T)

    shared = dict(gw_hi=gw_hi, gw_lo=gw_lo, w1=w1_dev, w2=w2_dev, b1r=b1r,
                  b2n=b2n, wo3=wo3, wfd3=wfd3, wfv3=wfv3, bor=bor)
    in_maps = []
    for i in range(NCORES):
        sl = slice(i * N, (i + 1) * N)
        m = dict(shared)
        m["xt_hi"] = np.ascontiguousarray(xt_hi[:, sl])
        m["xt_lo"] = np.ascontiguousarray(xt_lo[:, sl])
        in_maps.append(m)
    return in_maps


def run_on_device(in_maps, trace=False):
    from concourse.bass_utils import run_bass_kernel_spmd

    nc = _get_nc()
    return run_bass_kernel_spmd(nc, in_maps, list(range(NCORES)), trace=trace)


def assemble_outputs(results):
    f32 = np.float32
    assoc = np.empty((B, OD), f32)
    fb_d = np.empty((B, D), f32)
    fb_v = np.empty((B, V), f32)
    gate = np.empty((B, E), f32)
    for i, r in enumerate(results):
        sl = slice(i * N, (i + 1) * N)
        assoc[sl] = r["assocT"].T
        fb_d[sl] = r["fbdT"].T
        fb_v[sl] = r["fbvT"].T
        gate[sl] = r["gatew"]
    return assoc, fb_d, fb_v, gate


def kernel(**inputs):
    in_maps = prepare_in_maps(**inputs)
    res = run_on_device(in_maps, trace=False)
    return assemble_outputs(res.results)


# revision 14
# speedup vs baseline: 1.4692x; 1.4692x over previous
"""Trainium2 Bass kernel for nn_AssociationCortex (MoE routing block).

Math (per reference):
  x = [dorsal | ventral]                    (B, C)   B=8192, C=2048
  gate_logits = x @ gate_W                  (B, E)   E=8
  top-2 softmax -> gate_weights             (B, E)  (2 nonzeros/row)
  h_e = gelu(x @ W1_e + b1_e)               (B, ED)  ED=1024, all 8 experts
  out_e = h_e @ W2_e + b2_e                 (B, ED)
  bound = sum_e g_e * out_e                 (B, ED)
  assoc = bound @ Wo + bo                   (B, OD)
  fb_d = 0.5 * assoc @ Wfd ; fb_v = 0.5 * assoc @ Wfv
  returns (assoc, fb_d, fb_v, gate_weights)

Strategy: data-parallel over batch across 8 NeuronCores (1024 rows/core,
all params replicated; no collectives). On-device layout is transposed
(features on partitions, batch on the free axis) so every matmul uses the
natural weight block as the stationary operand and the activation as the
moving operand. Compute in bf16 (fp32 PSUM accumulation); gate logits use
a bf16 hi+lo decomposition (x_hi@g_hi + x_hi@g_lo + x_lo@g_hi) for ~fp32
accuracy, since top-k selection is discretely sensitive to logit error.
Gate weighting of h is done with a PE row-broadcast of the dense gate row
followed by a VectorE multiply. All host-side work is layout prep only
(transpose/cast/shard); every FLOP of the model runs on device.
"""

import sys

if "/opt/trn_rl_repo" not in sys.path:
    sys.path.insert(0, "/opt/trn_rl_repo")

import numpy as np
import ml_dtypes

import concourse.bass as bass
import concourse.mybir as mybir
import concourse.tile as tile
from concourse import bacc
from concourse.masks import make_identity



BF16 = mybir.dt.bfloat16
F32 = mybir.dt.float32
AF = mybir.ActivationFunctionType
ALU = mybir.AluOpType
bf16 = ml_dtypes.bfloat16

# Problem dims (hardcoded per spec)
B, D, V, E, ED, OD = 8192, 1024, 1024, 8, 1024, 1024
C = D + V
NCORES = 8
N = B // NCORES          # batch rows per core (1024)
BC = 512                 # batch chunk (matmul moving free dim)
NJ = N // BC             # chunks per core (2)
CT = C // 128            # 16 c tiles
DT8 = ED // 128          # 8 feature tiles
FB_STRENGTH = 0.5


def build_nc():
    nc = bacc.Bacc("TRN2", target_bir_lowering=False, debug=False, num_devices=NCORES)

    # ---- DRAM parameters (per-core shards / replicated) ----
    xt_hi = nc.declare_dram_parameter("xt_hi", [C, N], BF16, isOutput=False)
    xt_lo = nc.declare_dram_parameter("xt_lo", [C, N], BF16, isOutput=False)
    gw_hi = nc.declare_dram_parameter("gw_hi", [C, E], BF16, isOutput=False)
    gw_lo = nc.declare_dram_parameter("gw_lo", [C, E], BF16, isOutput=False)
    # w1[e, dblk, p, co, f] = W1[e, co*128+p, dblk*128+f]
    w1 = nc.declare_dram_parameter("w1", [E, DT8, 128, CT, 128], BF16, isOutput=False)
    # w2[e, fblk, p, do, f] = W2[e, do*128+p, fblk*128+f]
    w2 = nc.declare_dram_parameter("w2", [E, DT8, 128, DT8, 128], BF16, isOutput=False)
    # b1r[e, p, t] = b1[e, t*128+p]
    b1r = nc.declare_dram_parameter("b1r", [E, 128, DT8], F32, isOutput=False)
    b2n = nc.declare_dram_parameter("b2n", [E, ED], BF16, isOutput=False)
    # wo3[oblk, p, fo, o2] = Wo[fo*128+p, oblk*128+o2]; same scheme for wfd/wfv
    wo3 = nc.declare_dram_parameter("wo3", [DT8, 128, DT8, 128], BF16, isOutput=False)
    wfd3 = nc.declare_dram_parameter("wfd3", [DT8, 128, DT8, 128], BF16, isOutput=False)
    wfv3 = nc.declare_dram_parameter("wfv3", [DT8, 128, DT8, 128], BF16, isOutput=False)
    # bor[p, t] = bo[t*128+p]
    bor = nc.declare_dram_parameter("bor", [128, DT8], F32, isOutput=False)

    assocT = nc.declare_dram_parameter("assocT", [OD, N], F32, isOutput=True)
    fbdT = nc.declare_dram_parameter("fbdT", [D, N], F32, isOutput=True)
    fbvT = nc.declare_dram_parameter("fbvT", [V, N], F32, isOutput=True)
    gatew = nc.declare_dram_parameter("gatew", [N, E], F32, isOutput=True)

    from contextlib import ExitStack

    with ExitStack() as ctx:
        tc = ctx.enter_context(tile.TileContext(nc))
        pool = lambda name, bufs, **kw: ctx.enter_context(  # noqa: E731
            tc.tile_pool(name=name, bufs=bufs, **kw))
        constp = pool("const", 1)
        xhp = pool("xh", CT)
        xlp = pool("xl", 6)
        gwp = pool("gw", 2 * CT)
        gsp = pool("gsmall", 1)
        gnp = pool("gnat", 24)
        w1p = pool("w1d", 5)
        w2p = pool("w2f", 5)
        wfp = pool("wfin", 4)
        biasp = pool("bias", 4)
        gbp = pool("gb", 3)
        hbp = pool("hbf", 4)
        hgp = pool("hg", 34)
        boundp = pool("bound", 2 * DT8)
        evp = pool("evf32", 4)
        php = pool("ps_h", 3, space="PSUM")
        pzp = pool("ps_z", 3, space="PSUM")
        pmp = pool("ps_misc", 2, space="PSUM")
        if True:
            # ---- constants ----
            ident = constp.tile([128, 128], F32)
            make_identity(nc, ident[:])
            ones_bf = constp.tile([1, 128], BF16)
            nc.vector.memset(ones_bf[:], 1.0)

            # ---- load x^T (hi) tiles: persistent through the expert loop ----
            xh = []
            for c in range(CT):
                t = xhp.tile([128, N], BF16, tag="xh")
                nc.sync.dma_start(t[:], xt_hi[c * 128 : (c + 1) * 128, :])
                xh.append(t)

            # ---- gate logits, hi/lo decomposed, accumulated in PSUM ----
            # glog_ps[j] (8, BC) += gwh[c].T@xh + gwl[c].T@xh + gwh[c].T@xl
            glog_ps = [pmp.tile([E, BC], F32, tag="pm", name=f"glog{_}") for _ in range(NJ)]
            gwh, gwl = [], []
            for c in range(CT):
                th = gwp.tile([128, E], BF16, tag="gwh")
                nc.sync.dma_start(th[:], gw_hi[c * 128 : (c + 1) * 128, :])
                gwh.append(th)
                tl = gwp.tile([128, E], BF16, tag="gwl")
                nc.sync.dma_start(tl[:], gw_lo[c * 128 : (c + 1) * 128, :])
                gwl.append(tl)
            for c in range(CT):
                xlt = xlp.tile([128, N], BF16, tag="xl")
                nc.sync.dma_start(xlt[:], xt_lo[c * 128 : (c + 1) * 128, :])
                first, last = c == 0, c == CT - 1
                # same stationary serves adjacent matmuls (LDW reuse)
                for j in range(NJ):
                    bs = slice(j * BC, (j + 1) * BC)
                    nc.tensor.matmul(glog_ps[j][:], gwh[c][:], xh[c][:, bs],
                                     start=first, stop=False)
                for j in range(NJ):
                    bs = slice(j * BC, (j + 1) * BC)
                    nc.tensor.matmul(glog_ps[j][:], gwh[c][:], xlt[:, bs],
                                     start=False, stop=False)
                for j in range(NJ):
                    bs = slice(j * BC, (j + 1) * BC)
                    nc.tensor.matmul(glog_ps[j][:], gwl[c][:], xh[c][:, bs],
                                     start=False, stop=last)

            g_sb = gsp.tile([E, N], F32)
            for j in range(NJ):
                nc.vector.tensor_copy(g_sb[:, j * BC : (j + 1) * BC], glog_ps[j][:])

            # ---- top-2 + softmax, in natural layout (batch on partitions) ----
            gw_nat = []
            for t in range(N // 128):
                tr_ps = pmp.tile([128, E], F32, tag="pm")
                nc.tensor.transpose(tr_ps[:], g_sb[:, t * 128 : (t + 1) * 128],
                                    ident[:E, :E])
                gn = gnp.tile([128, E], F32, tag="gn")
                nc.vector.tensor_copy(gn[:], tr_ps[:])

                m1 = gnp.tile([128, 1], F32, tag="m")
                nc.vector.tensor_reduce(m1[:], gn[:], mybir.AxisListType.X, ALU.max)
                mask1 = gnp.tile([128, E], F32, tag="gn")
                nc.vector.tensor_scalar(mask1[:], gn[:], m1[:], None, ALU.is_equal)
                masked = gnp.tile([128, E], F32, tag="gn")
                nc.vector.scalar_tensor_tensor(masked[:], mask1[:], -1e30, gn[:],
                                               ALU.mult, ALU.add)
                m2 = gnp.tile([128, 1], F32, tag="m")
                nc.vector.tensor_reduce(m2[:], masked[:], mybir.AxisListType.X, ALU.max)
                mask2 = gnp.tile([128, E], F32, tag="gn")
                nc.vector.tensor_scalar(mask2[:], masked[:], m2[:], None, ALU.is_equal)
                negm2 = gnp.tile([128, 1], F32, tag="m")
                nc.vector.tensor_scalar_mul(negm2[:], m2[:], -1.0)
                w1v = gnp.tile([128, 1], F32, tag="m")
                nc.scalar.activation(w1v[:], m1[:], AF.Sigmoid, bias=negm2[:])
                w2v = gnp.tile([128, 1], F32, tag="m")
                nc.vector.tensor_scalar(w2v[:], w1v[:], -1.0, 1.0, ALU.mult, ALU.add)

                gwn = gnp.tile([128, E], F32, tag="gn")
                nc.vector.tensor_scalar(gwn[:], mask2[:], w2v[:], None, ALU.mult)
                nc.vector.scalar_tensor_tensor(gwn[:], mask1[:], w1v[:], gwn[:],
                                               ALU.mult, ALU.add)
                nc.sync.dma_start(gatew[t * 128 : (t + 1) * 128, :], gwn[:])
                gw_nat.append(gwn)

            # ---- dense gate rows g^T (E, N) in bf16 ----
            gT_bf = gsp.tile([E, N], BF16)
            for j in range(NJ):
                gT_ps = pmp.tile([E, BC], F32, tag="pm")
                for k in range(BC // 128):
                    t = j * (BC // 128) + k
                    nc.tensor.transpose(gT_ps[:, k * 128 : (k + 1) * 128],
                                        gw_nat[t][:], ident[:])
                nc.vector.tensor_copy(gT_bf[:, j * BC : (j + 1) * BC], gT_ps[:])

            # per-expert gate rows at partition 0 (matmul rhs needs base 0)
            gTrow = []
            for e in range(E):
                gr = gsp.tile([1, N], BF16, tag=f"gtr{e}", name=f"gtr{e}")
                nc.sync.dma_start(gr[:], gT_bf[e : e + 1, :])
                gTrow.append(gr)

            # ---- seed bound with sum_e g_e * b2_e  (PE over e-dim) ----
            b2sb = gsp.tile([E, ED], BF16)
            nc.sync.dma_start(b2sb[:], b2n[:, :])
            bound = [[None] * DT8 for _ in range(NJ)]
            for f in range(DT8):
                sps = [pzp.tile([128, BC], F32, tag="pz", name=f"sps{f}_{j}")
                       for j in range(NJ)]
                for j in range(NJ):
                    bs = slice(j * BC, (j + 1) * BC)
                    nc.tensor.matmul(sps[j][:], b2sb[:, f * 128 : (f + 1) * 128],
                                     gT_bf[:, bs], start=True, stop=True)
                for j in range(NJ):
                    bt = boundp.tile([128, BC], F32, tag="bound",
                                     name=f"bound{f}_{j}")
                    nc.vector.tensor_copy(bt[:], sps[j][:])
                    bound[j][f] = bt

            # ---- expert loop ----
            for e in range(E):
                # broadcast gate row e -> (128, N) bf16
                gb = gbp.tile([128, N], BF16, tag="gb")
                for j in range(NJ):
                    gb_ps = pmp.tile([128, BC], F32, tag="pm")
                    nc.tensor.matmul(gb_ps[:], ones_bf[:],
                                     gTrow[e][:, j * BC : (j + 1) * BC],
                                     start=True, stop=True)
                    nc.vector.tensor_copy(gb[:, j * BC : (j + 1) * BC], gb_ps[:])

                b1sb = biasp.tile([128, DT8], F32, tag="b1")
                nc.sync.dma_start(b1sb[:], b1r[e, :, :])

                hg = [[None] * NJ for _ in range(DT8)]
                for d in range(DT8):
                    w1t = w1p.tile([128, CT, 128], BF16, tag="w1")
                    nc.sync.dma_start(w1t[:], w1[e, d, :, :, :])
                    ph = [php.tile([128, BC], F32, tag="ph", name=f"ph{e}_{d}_{j}")
                          for j in range(NJ)]
                    for c in range(CT):
                        for j in range(NJ):
                            bs = slice(j * BC, (j + 1) * BC)
                            nc.tensor.matmul(ph[j][:], w1t[:, c, :], xh[c][:, bs],
                                             start=(c == 0), stop=(c == CT - 1))
                    for j in range(NJ):
                        bs = slice(j * BC, (j + 1) * BC)
                        hb = hbp.tile([128, BC], BF16, tag="hb")
                        nc.scalar.activation(hb[:], ph[j][:], AF.Gelu,
                                             bias=b1sb[:, d : d + 1])
                        ht = hgp.tile([128, BC], BF16, tag="hg")
                        nc.vector.tensor_mul(ht[:], hb[:], gb[:, bs])
                        hg[d][j] = ht

                for f in range(DT8):
                    w2t = w2p.tile([128, DT8, 128], BF16, tag="w2")
                    nc.sync.dma_start(w2t[:], w2[e, f, :, :, :])
                    pz = [pzp.tile([128, BC], F32, tag="pz", name=f"pz{e}_{f}_{j}")
                          for j in range(NJ)]
                    for d in range(DT8):
                        for j in range(NJ):
                            nc.tensor.matmul(pz[j][:], w2t[:, d, :], hg[d][j][:],
                                             start=(d == 0), stop=(d == DT8 - 1))
                    for j in range(NJ):
                        nc.vector.tensor_add(bound[j][f][:], pz[j][:], bound[j][f][:])

            # ---- final projections ----
            borsb = biasp.tile([128, DT8], F32, tag="bo")
            nc.sync.dma_start(borsb[:], bor[:, :])

            bound_bf = [[None] * DT8 for _ in range(NJ)]
            for j in range(NJ):
                for f in range(DT8):
                    bb = hgp.tile([128, BC], BF16, tag="hg")
                    nc.vector.tensor_copy(bb[:], bound[j][f][:])
                    bound_bf[j][f] = bb

            assoc_bf = [[None] * NJ for _ in range(DT8)]
            for o in range(DT8):
                wot = wfp.tile([128, DT8, 128], BF16, tag="wf")
                nc.sync.dma_start(wot[:], wo3[o, :, :, :])
                pa = [pzp.tile([128, BC], F32, tag="pz", name=f"pa{o}_{j}")
                      for j in range(NJ)]
                for f in range(DT8):
                    for j in range(NJ):
                        nc.tensor.matmul(pa[j][:], wot[:, f, :], bound_bf[j][f][:],
                                         start=(f == 0), stop=(f == DT8 - 1))
                for j in range(NJ):
                    af = evp.tile([128, BC], F32, tag="ev")
                    nc.vector.tensor_scalar(af[:], pa[j][:], borsb[:, o : o + 1],
                                            None, ALU.add)
                    ab = hgp.tile([128, BC], BF16, tag="hg")
                    nc.vector.tensor_copy(ab[:], af[:])
                    assoc_bf[o][j] = ab
                    nc.sync.dma_start(
                        assocT[o * 128 : (o + 1) * 128, j * BC : (j + 1) * BC], af[:])

            for wi, (wext, out_ext) in enumerate(((wfd3, fbdT), (wfv3, fbvT))):
                for dd in range(DT8):
                    wt = wfp.tile([128, DT8, 128], BF16, tag="wf")
                    nc.sync.dma_start(wt[:], wext[dd, :, :, :])
                    pf = [pzp.tile([128, BC], F32, tag="pz",
                                   name=f"pf{wi}_{dd}_{j}")
                          for j in range(NJ)]
                    for o in range(DT8):
                        for j in range(NJ):
                            nc.tensor.matmul(pf[j][:], wt[:, o, :],
                                             assoc_bf[o][j][:],
                                             start=(o == 0), stop=(o == DT8 - 1))
                    for j in range(NJ):
                        ff = evp.tile([128, BC], F32, tag="ev")
                        nc.vector.tensor_scalar_mul(ff[:], pf[j][:], FB_STRENGTH)
                        nc.sync.dma_start(
                            out_ext[dd * 128 : (dd + 1) * 128,
                                    j * BC : (j + 1) * BC], ff[:])

    nc.compile()
    return nc


_NC_CACHE = []


def _get_nc():
    if not _NC_CACHE:
        _NC_CACHE.append(build_nc())
    return _NC_CACHE[0]


def _split_hi_lo(a32):
    hi = a32.astype(bf16)
    lo = (a32 - hi.astype(np.float32)).astype(bf16)
    return hi, lo


def prepare_in_maps(dorsal, ventral, gate_W, W1, b1, W2, b2, Wo, bo, Wfd, Wfv):
    f32 = np.float32
    x = np.concatenate([np.asarray(dorsal, f32), np.asarray(ventral, f32)], axis=1)
    xT = np.ascontiguousarray(x.T)  # (C, B)
    xt_hi, xt_lo = _split_hi_lo(xT)
    gw_hi, gw_lo = _split_hi_lo(np.asarray(gate_W, f32))

    W1 = np.asarray(W1, f32)
    W2 = np.asarray(W2, f32)
    w1_dev = np.ascontiguousarray(
        W1.reshape(E, CT, 128, DT8, 128).transpose(0, 3, 2, 1, 4)).astype(bf16)
    w2_dev = np.ascontiguousarray(
        W2.reshape(E, DT8, 128, DT8, 128).transpose(0, 3, 2, 1, 4)).astype(bf16)
    b1r = np.ascontiguousarray(
        np.asarray(b1, f32).reshape(E, DT8, 128).transpose(0, 2, 1))
    b2n = np.asarray(b2, f32).astype(bf16)

    def fin(w):
        return np.ascontiguousarray(
            np.asarray(w, f32).reshape(DT8, 128, DT8, 128).transpose(2, 1, 0, 3)
        ).astype(bf16)

    wo3, wfd3, wfv3 = fin(Wo), fin(Wfd), fin(Wfv)
    bor = np.ascontiguousarray(np.asarray(bo, f32).reshape(DT8, 128).T)

    shared = dict(gw_hi=gw_hi, gw_lo=gw_lo, w1=w1_dev, w2=w2_dev, b1r=b1r,
                  b2n=b2n, wo3=wo3, wfd3=wfd3, wfv3=wfv3, bor=bor)
    in_maps = []
    for i in range(NCORES):
        sl = slice(i * N, (i + 1) * N)
        m = dict(shared)
        m["xt_hi"] = np.ascontiguousarray(xt_hi[:, sl])
        m["xt_lo"] = np.ascontiguousarray(xt_lo[:, sl])
        in_maps.append(m)
    return in_maps


def run_on_device(in_maps, trace=False):
    from concourse.bass_utils import run_bass_kernel_spmd

    nc = _get_nc()
    return run_bass_kernel_spmd(nc, in_maps, list(range(NCORES)), trace=trace)


def assemble_outputs(results):
    f32 = np.float32
    assoc = np.empty((B, OD), f32)
    fb_d = np.empty((B, D), f32)
    fb_v = np.empty((B, V), f32)
    gate = np.empty((B, E), f32)
    for i, r in enumerate(results):
        sl = slice(i * N, (i + 1) * N)
        assoc[sl] = r["assocT"].T
        fb_d[sl] = r["fbdT"].T
        fb_v[sl] = r["fbvT"].T
        gate[sl] = r["gatew"]
    return assoc, fb_d, fb_v, gate


def kernel(**inputs):
    in_maps = prepare_in_maps(**inputs)
    res = run_on_device(in_maps, trace=False)
    return assemble_outputs(res.results)
